# revision 19
# baseline (speedup 1.0000x reference)
"""AxialAttention (axis=height) Trainium2 Bass kernel, v2.

Per-core work: one (b,t) pair = 128 lines (w) of length L=H=128, C=256, 8
heads x 32. Rewritten from the v1 baseline (308us) around engine balance:

  - QK projection as fp8e4 DoubleRow matmuls (K=256 in one pass, 0.5
    cyc/col); scores as fp8 DoubleRow with a zero second weight-tile
    (K=32 streams at 0.5 cyc/col).
  - The rel_bias multiply stays on DVE (bf16 2x mode); exp on Act with
    SCALE folded into the activation scale.
  - PSUM->SBUF copies (qk fp8, V bf16, out bf16) spread across
    Pool/DVE/Act round-robin; biases ride the copies (per-partition).
  - Y transpose via DMA-crossbar transpose (dma_start_transpose), not PE.
  - Softmax denominators via N=1 ones-matmuls into spare PSUM cols;
    attention Y written back into the freed score PSUM tile.
  - Output projection per 32-line quarter, overlapped; out stored bf16
    in a flat DRAM layout, host reassembles + upcasts.

PSUM: 2x score tiles [128,1024] (4 banks) + 2x stage tiles [128,1024]
(4 banks, shared by stage-A / V / out-proj chunks).
"""

import os

import numpy as np
import ml_dtypes

import concourse.bacc as bacc
import concourse.mybir as mybir
from concourse import tile
from concourse.bass import broadcast_tensor_aps
from concourse.bass_utils import run_bass_kernel_spmd

BF16 = ml_dtypes.bfloat16
F8 = ml_dtypes.float8_e4m3fn

B, T, C, H, W = 2, 4, 256, 128, 128
HEADS, DH = 8, 32
SCALE = DH ** (-0.5)
DT_B = mybir.dt.bfloat16
DT_F = mybir.dt.float32
DT_8 = mybir.dt.float8e4
AF = mybir.ActivationFunctionType
ALU = mybir.AluOpType
PM = mybir.MatmulPerfMode

HEADSEQ = [0, 4, 1, 5, 2, 6, 3, 7]       # head order along v/eb/yt feature axis
QUADS = [[0, 4, 1, 5], [2, 6, 3, 7]]     # heads per score tile
SEQIDX = {h: s for s, h in enumerate(HEADSEQ)}

NWB = 8          # w-blocks of 16 lines
WBLK = 16

# copy-engine schedules (tunable): entries are "v" (DVE), "s" (Act), "g" (Pool)
# PSUM sources: DVE/Act only (GPSIMD cannot access PSUM on real TRN2)
SPLIT_A = ["v", "s"] * 64
SPLIT_B = ["v"] * 64
# out-proj evacs are lag work: keep only ~5/64 on Act to balance engine busy.
# First 64 entries: identical cycling pattern to the balanced config; last 6
# (the epilogue, after Act's final exp) go to the otherwise-idle Act.
SPLIT_G = [(["v"] * 6 + ["s"] + ["v"] * 6)[i % 13] for i in range(64)] + ["s"] * 6
ENG_E = "v"
ENG_D = "g"


def build_program():
    nc = bacc.Bacc("TRN2")

    xw_d = nc.dram_tensor("xw_d", [128, 2, W, H], DT_B, kind="ExternalInput")
    x8_d = nc.dram_tensor("x8_d", [128, 2, W, H], DT_8, kind="ExternalInput")
    cb8_d = nc.dram_tensor("cb8_d", [128, 1024], DT_8, kind="ExternalInput")
    cb16_d = nc.dram_tensor("cb16_d", [128, 2048], DT_B, kind="ExternalInput")
    cf32_d = nc.dram_tensor("cf32_d", [128, 8], DT_F, kind="ExternalInput")
    out_d = nc.dram_tensor("out_d", [32, 2, 128, 512], DT_B, kind="ExternalOutput")

    eng = {"v": nc.vector, "s": nc.scalar, "g": nc.gpsimd}

    def copy_with_bias(e, dst, src, bias):
        if e == "s":
            nc.scalar.activation(dst, src, AF.Identity, bias=bias)
        else:
            eng[e].tensor_scalar(dst, src, bias, None, ALU.add)

    with tile.TileContext(nc) as tc:
        with (
            tc.tile_pool(name="const", bufs=1) as cpool,
            tc.tile_pool(name="xin", bufs=1) as xpool,
            tc.tile_pool(name="qk8", bufs=1) as qkpool,
            tc.tile_pool(name="vt", bufs=1) as vpool,
            tc.tile_pool(name="ex", bufs=4) as epool,
            tc.tile_pool(name="aw", bufs=4) as apool,
            tc.tile_pool(name="inv", bufs=4) as ipool,
            tc.tile_pool(name="yn", bufs=1) as ynpool,
            tc.tile_pool(name="yt", bufs=1) as ytpool,
            tc.tile_pool(name="ot", bufs=4) as otpool,
            tc.tile_pool(name="psc", bufs=2, space="PSUM") as scpool,
            tc.tile_pool(name="pst", bufs=2, space="PSUM") as stpool,
            tc.tile_pool(name="psy", bufs=1, space="PSUM") as ypool,
        ):
            # ---- constants ----
            cb8 = cpool.tile([128, 1024], DT_8, tag="cb8")
            nc.sync.dma_start(out=cb8[:], in_=cb8_d[:])
            cf = cpool.tile([128, 8], DT_F, tag="cf32")
            nc.sync.dma_start(out=cf[:], in_=cf32_d[:])
            cb16 = cpool.tile([128, 2048], DT_B, tag="cb16")
            ones = cpool.tile([128, 1], DT_B, tag="ones")
            nc.vector.memset(ones[:], 1.0)
            warm = cpool.tile([128, 1], DT_B, tag="warm")
            nc.scalar.activation(warm[:], ones[:], AF.Exp)

            w8 = cb8[:].rearrange("p (a m) -> p a m", a=2)          # [128,2,512]
            wv = cb16[:, 0:512].rearrange("p (a f) -> p a f", a=2)  # [128,2,256]
            wo = cb16[:, 512:1024].rearrange("p (a f) -> p a f", a=2)
            eb = cb16[:, 1024:2048]                                  # [m,(s,l)]

            # ---- persistent double-buffered tiles ----
            xw_t = [xpool.tile([128, 2, WBLK, H], DT_B, tag=f"xw{i}", name=f"xw{i}") for i in range(2)]
            x8_t = [xpool.tile([128, 2, WBLK, H], DT_8, tag=f"x8{i}", name=f"x8{i}") for i in range(2)]
            q8_t = [qkpool.tile([128, 2, WBLK, 128], DT_8, tag=f"q8{i}", name=f"q8{i}") for i in range(2)]
            k8_t = [qkpool.tile([128, 2, WBLK, 2, 128], DT_8, tag=f"k8{i}", name=f"k8{i}") for i in range(2)]
            nc.vector.memset(k8_t[0][:, :, :, 1, :].bitcast(mybir.dt.uint16), 0)
            nc.gpsimd.memset(k8_t[1][:, :, :, 1, :].bitcast(mybir.dt.uint16), 0)
            # V tiles: one per 4-line group, double buffered: [m, (j4, s, d)]
            v_t = [vpool.tile([128, 1024], DT_B, tag=f"v{i}", name=f"v{i}") for i in range(8)]
            yn_t = [ynpool.tile([128, 2, 4, 128], DT_B, tag=f"yn{i}", name=f"yn{i}") for i in range(4)]
            yt = ytpool.tile([128, 2, H * W], DT_B, tag="yt")

            def x_load8(wb):
                i = wb % 2
                nc.sync.dma_start(
                    out=x8_t[i][:], in_=x8_d[:, :, wb * WBLK:(wb + 1) * WBLK, :]
                )

            def x_loadw(wb):
                i = wb % 2
                nc.sync.dma_start(
                    out=xw_t[i][:], in_=xw_d[:, :, wb * WBLK:(wb + 1) * WBLK, :]
                )

            def x_load(wb):
                x_load8(wb)
                x_loadw(wb)

            # wb0 split into head (w0..4, gates pair 0) + rest
            nc.sync.dma_start(out=x8_t[0][:, :, 0:4, :], in_=x8_d[:, :, 0:4, :])
            nc.sync.dma_start(out=xw_t[0][:, :, 0:4, :], in_=xw_d[:, :, 0:4, :])
            nc.sync.dma_start(out=cb16[:], in_=cb16_d[:])
            nc.sync.dma_start(out=x8_t[0][:, :, 4:16, :], in_=x8_d[:, :, 4:16, :])
            nc.sync.dma_start(out=xw_t[0][:, :, 4:16, :], in_=xw_d[:, :, 4:16, :])
            x_load(1)

            na, nb, ng = [0], [0], [0]

            def stage_a_chunk(wb, idx):
                # idx 0..15 -> (ft, ch)
                ib = wb % 2
                ch, ft = divmod(idx, 4)
                ps = stpool.tile([128, 512], DT_F, tag="st", name="st_a")
                nc.tensor.matmul(
                    ps[:],
                    lhsT=w8[:, :, ft * 128:(ft + 1) * 128],
                    rhs=x8_t[ib][:, :, ch * 4:(ch + 1) * 4, :],
                    start=True, stop=True,
                    perf_mode=PM.DoubleRow,
                )
                if ft < 2:
                    dst = q8_t[ib][:, ft, ch * 4:(ch + 1) * 4, :]
                else:
                    dst = k8_t[ib][:, ft - 2, ch * 4:(ch + 1) * 4, 0, :]
                e = SPLIT_A[na[0] % len(SPLIT_A)]
                na[0] += 1
                copy_with_bias(e, dst, ps[:], cf[:, ft:ft + 1])

            def v_chunk(wb, idx):
                # idx 0..7 -> 2 lines
                ib = wb % 2
                vq, j2 = divmod(idx, 2)
                vt = v_t[ib * 4 + vq]
                ps = stpool.tile([128, 512], DT_F, tag="st", name="st_v")
                for j in range(2):
                    for a in range(2):
                        nc.tensor.matmul(
                            ps[:, j * 256:(j + 1) * 256],
                            lhsT=xw_t[ib][:, a, vq * 4 + j2 * 2 + j, :],
                            rhs=wv[:, a, :],
                            start=(a == 0), stop=(a == 1),
                        )
                e = SPLIT_B[nb[0] % len(SPLIT_B)]
                nb[0] += 1
                dst = vt[:, j2 * 512:(j2 + 1) * 512]
                if e == "s":
                    nc.scalar.activation(dst, ps[:], AF.Copy)
                else:
                    eng[e].tensor_copy(dst, ps[:])

            def oproj_chunk(qw, ct, half=None):
                # 4-line group qw (w = qw*4..+4), full h; one chunk per ct.
                # half=0/1: 2-line subchunk (used to shorten the tail).
                if half is None:
                    w0, nw, cols = qw * 4, 4, 512
                else:
                    w0, nw, cols = qw * 4 + half * 2, 2, 256
                ot = otpool.tile([128, 512], DT_B, tag=f"ot{ct}", name="ot")
                po = stpool.tile([128, 512], DT_F, tag="st", name="st_o")
                for a in range(2):
                    rhs = yt[:, a, :].rearrange(
                        "p (w h) -> p w h", h=H
                    )[:, w0:w0 + nw, :]
                    nc.tensor.matmul(
                        po[:, 0:cols],
                        lhsT=wo[:, a, ct * 128:(ct + 1) * 128],
                        rhs=rhs,
                        start=(a == 0), stop=(a == 1),
                    )
                e = SPLIT_G[ng[0] % len(SPLIT_G)]
                ng[0] += 1
                copy_with_bias(e, ot[:, 0:cols], po[:, 0:cols], cf[:, 4 + ct:5 + ct])
                if half is None:
                    nc.sync.dma_start(out=out_d[qw, ct], in_=ot[:])
                else:
                    nc.sync.dma_start(
                        out=out_d[qw, ct][:, half * 256:half * 256 + 256],
                        in_=ot[:, 0:256],
                    )

            def attention_pair(wb, pj):
                ib = wb % 2
                wloc = pj * 2
                sc0 = scpool.tile([128, 1024], DT_F, tag="sc", name="sc0")
                sc1 = scpool.tile([128, 1024], DT_F, tag="sc", name="sc1")
                sc = [sc0, sc1]
                attnws = []
                for tq in range(2):
                    st = sc[tq]
                    for j in range(2):
                        for qi in range(4):
                            h = QUADS[tq][qi]
                            b = h % 4
                            hh = h // 4
                            lhsT = k8_t[ib][32 * b:32 * b + 32, hh, wloc + j, :, :]
                            rhs = q8_t[ib][
                                32 * b:32 * b + 32, hh:hh + 1, wloc + j, :
                            ].broadcast_to((32, 2, 128))
                            col = (qi // 2) * 512 + (qi % 2) * 256 + j * 128
                            nc.tensor.matmul(
                                st[:, col:col + 128],
                                lhsT=lhsT, rhs=rhs,
                                start=True, stop=True,
                                perf_mode=PM.DoubleRow,
                                tile_position=(32 * b, 0),
                            )
                    exps = epool.tile([128, 1024], DT_B, tag="exps", name="exps")
                    nc.scalar.activation(exps[:], st[:], AF.Exp, scale=SCALE)
                    attnw = apool.tile([128, 1024], DT_B, tag="attnw", name="attnw")
                    e5 = exps[:].rearrange("p (s j l) -> p s j l", s=4, j=2)
                    eb5 = eb[:, tq * 512:(tq + 1) * 512].rearrange(
                        "p (s l) -> p s l", s=4
                    )[:, :, None, :]
                    i0, i1 = broadcast_tensor_aps(e5, eb5)
                    aw5 = attnw[:].rearrange("p (s j l) -> p s j l", s=4, j=2)
                    eng[ENG_D].tensor_tensor(aw5, i0, i1, ALU.mult)
                    attnws.append(attnw)

                y_ps = ypool.tile([128, 528], DT_F, tag="y", name="y_ps")
                vt = v_t[ib * 4 + pj // 2]
                jj0 = (pj % 2) * 2
                for tq in range(2):
                    attnw = attnws[tq]
                    for j in range(2):
                        for qi in range(4):
                            s = tq * 4 + qi
                            col = (qi // 2) * 512 + (qi % 2) * 256 + j * 128
                            asl = attnw[:, col:col + 128]
                            nc.tensor.matmul(
                                y_ps[:, j * 256 + s * 32:j * 256 + s * 32 + 32],
                                lhsT=asl,
                                rhs=vt[:, (jj0 + j) * 256 + s * 32:(jj0 + j) * 256 + s * 32 + 32],
                                start=True, stop=True,
                            )
                            nc.tensor.matmul(
                                y_ps[:, 512 + j * 8 + s:512 + j * 8 + s + 1],
                                lhsT=asl, rhs=ones[:],
                                start=True, stop=True,
                            )

                invd = ipool.tile([128, 16], DT_F, tag="invd", name="invd")
                nc.vector.reciprocal(invd[:], y_ps[:, 512:528])

                ynt = yn_t[(wb * 4 + pj // 2) % 4]
                y5 = y_ps[:, 0:512].rearrange(
                    "p (j ch s2 d) -> p j ch s2 d", j=2, ch=2, s2=4
                )
                iv5 = invd[:].rearrange("p (j ch s2) -> p j ch s2", j=2, ch=2)[
                    :, :, :, :, None
                ]
                out5 = ynt[:, :, (pj % 2) * 2:(pj % 2) * 2 + 2, :].rearrange(
                    "p ch j (s2 d) -> p j ch s2 d", s2=4
                )
                i0, i1 = broadcast_tensor_aps(y5, iv5)
                eng[ENG_E].tensor_tensor(out5, i0, i1, ALU.mult)

                last_grp = (wb == NWB - 1 and pj >= 6)
                if last_grp:
                    # 2-line transpose per pair so the tail drains sooner
                    lb = wb * WBLK + pj * 2
                    for chh in range(2):
                        srcv = ynt[:, chh, (pj % 2) * 2:(pj % 2) * 2 + 2, :]
                        dstv = yt[:, chh, :].rearrange(
                            "p (w h) -> p w h", h=H
                        )[:, lb:lb + 2, :]
                        nc.sync.dma_start_transpose(dstv, srcv)
                elif pj % 2 == 1:
                    lb = wb * WBLK + (pj // 2) * 4
                    for chh in range(2):
                        srcv = ynt[:, chh, :, :]
                        dstv = yt[:, chh, :].rearrange(
                            "p (w h) -> p w h", h=H
                        )[:, lb:lb + 4, :]
                        nc.sync.dma_start_transpose(dstv, srcv)

            # ---- software-pipelined emission ----
            # prologue: wb0 projections
            for i in range(16):
                stage_a_chunk(0, i)
            for i in range(8):
                v_chunk(0, i)

            for wb in range(NWB):
                for pj in range(8):
                    if pj == 0 and wb + 2 < NWB:
                        x_load8(wb + 2)
                    if pj == 2 and wb + 2 < NWB:
                        x_loadw(wb + 2)
                    if wb + 1 < NWB:
                        stage_a_chunk(wb + 1, pj * 2)
                        stage_a_chunk(wb + 1, pj * 2 + 1)
                        v_chunk(wb + 1, pj)
                    attention_pair(wb, pj)
                    g = wb * 8 + pj          # global pair index
                    if g >= 2:
                        oproj_chunk((g - 2) // 2, (g - 2) % 2)
            # epilogue: last group's projection, 2-line granularity
            oproj_chunk(31, 0, half=0)
            oproj_chunk(31, 1, half=0)
            oproj_chunk(31, 0, half=1)
            oproj_chunk(31, 1, half=1)

    nc.compile()
    return nc


_NC = None


def _get_nc():
    global _NC
    if _NC is None:
        _NC = build_program()
    return _NC


def _prep_small(rel_bias, Wqkv, bqkv, Wout, bout):
    rel_bias = np.asarray(rel_bias, np.float32)
    Wqkv = np.asarray(Wqkv, np.float32)
    bqkv = np.asarray(bqkv, np.float32)
    Wout = np.asarray(Wout, np.float32)
    bout = np.asarray(bout, np.float32)

    perm_v = np.concatenate([np.arange(32) + h * 32 for h in HEADSEQ])

    # cb8: qk weights as DoubleRow lhsT [p,(a,m)], features in natural order
    w8 = Wqkv[:, 0:512].reshape(2, 128, 512).transpose(1, 0, 2).reshape(128, 1024)
    cb8 = np.ascontiguousarray(w8).astype(F8)

    # cb16: [wv 512 | wo 512 | eb 1024]
    wv = Wqkv[:, 512:768][:, perm_v].reshape(2, 128, 256).transpose(1, 0, 2)
    wo = Wout[perm_v, :].reshape(2, 128, 256).transpose(1, 0, 2)
    expbt = np.exp(rel_bias.transpose(0, 2, 1))  # [hd, m, l]
    ebs = expbt[HEADSEQ].transpose(1, 0, 2).reshape(128, 1024)  # [m,(s,l)]
    cb16 = np.concatenate(
        [wv.reshape(128, 512), wo.reshape(128, 512), ebs], axis=1
    ).astype(BF16)

    bout2 = bout + bqkv[512:768] @ Wout
    cf32 = np.stack(
        [
            bqkv[0:128], bqkv[128:256], bqkv[256:384], bqkv[384:512],
            bout2[0:128], bout2[128:256],
            np.zeros(128, np.float32), np.zeros(128, np.float32),
        ],
        axis=1,
    ).astype(np.float32)

    return {
        "cb8_d": np.ascontiguousarray(cb8),
        "cb16_d": np.ascontiguousarray(cb16),
        "cf32_d": np.ascontiguousarray(cf32),
    }


def _prep_x(x_bt):
    # x_bt [C,H,W] f32 -> [p, a, w, h]
    xt = x_bt.reshape(2, 128, H, W).transpose(1, 0, 3, 2)
    xt = np.ascontiguousarray(xt)
    return xt.astype(BF16), xt.astype(F8)


def _unpack_out(arr):
    # arr [4, 2, 128, 4096] bf16 -> [C, H, W] f32
    a = np.asarray(arr).astype(np.float32).reshape(32, 2, 128, 4, 128)
    # [qw, ct, p, wl, h] -> [(ct,p)=c, h, (qw,wl)=w]
    return a.transpose(1, 2, 4, 0, 3).reshape(C, H, W)


def _run(x, rel_bias, Wqkv, bqkv, Wout, bout, **spmd_kwargs):
    x = np.asarray(x, dtype=np.float32)
    small = _prep_small(rel_bias, Wqkv, bqkv, Wout, bout)
    nc = _get_nc()
    core_ids = list(range(8))
    in_maps = []
    for i in core_ids:
        b, t = divmod(i, T)
        xw, x8 = _prep_x(x[b, t])
        m = dict(small)
        m["xw_d"] = xw
        m["x8_d"] = x8
        in_maps.append(m)
    res = run_bass_kernel_spmd(nc, in_maps, core_ids, **spmd_kwargs)
    out = np.empty((B, T, C, H, W), np.float32)
    for i in core_ids:
        b, t = divmod(i, T)
        out[b, t] = _unpack_out(res.results[i]["out_d"])
    return out, res


def kernel(x, rel_bias, Wqkv, bqkv, Wout, bout):
    out, _ = _run(x, rel_bias, Wqkv, bqkv, Wout, bout)
    return out



# revision 20
# speedup vs baseline: 1.0048x; 1.0048x over previous
"""AxialAttention (axis=height) Trainium2 Bass kernel, v2.

Per-core work: one (b,t) pair = 128 lines (w) of length L=H=128, C=256, 8
heads x 32. Rewritten from the v1 baseline (308us) around engine balance:

  - QK projection as fp8e4 DoubleRow matmuls (K=256 in one pass, 0.5
    cyc/col); scores as fp8 DoubleRow with a zero second weight-tile
    (K=32 streams at 0.5 cyc/col).
  - The rel_bias multiply stays on DVE (bf16 2x mode); exp on Act with
    SCALE folded into the activation scale.
  - PSUM->SBUF copies (qk fp8, V bf16, out bf16) spread across
    Pool/DVE/Act round-robin; biases ride the copies (per-partition).
  - Y transpose via DMA-crossbar transpose (dma_start_transpose), not PE.
  - Softmax denominators via N=1 ones-matmuls into spare PSUM cols;
    attention Y written back into the freed score PSUM tile.
  - Output projection per 32-line quarter, overlapped; out stored bf16
    in a flat DRAM layout, host reassembles + upcasts.

PSUM: 2x score tiles [128,1024] (4 banks) + 2x stage tiles [128,1024]
(4 banks, shared by stage-A / V / out-proj chunks).
"""

import os

import numpy as np
import ml_dtypes

import concourse.bacc as bacc
import concourse.mybir as mybir
from concourse import tile
from concourse.bass import broadcast_tensor_aps
from concourse.bass_utils import run_bass_kernel_spmd

BF16 = ml_dtypes.bfloat16
F8 = ml_dtypes.float8_e4m3fn

B, T, C, H, W = 2, 4, 256, 128, 128
HEADS, DH = 8, 32
SCALE = DH ** (-0.5)
DT_B = mybir.dt.bfloat16
DT_F = mybir.dt.float32
DT_8 = mybir.dt.float8e4
AF = mybir.ActivationFunctionType
ALU = mybir.AluOpType
PM = mybir.MatmulPerfMode

HEADSEQ = [0, 4, 1, 5, 2, 6, 3, 7]       # head order along v/eb/yt feature axis
QUADS = [[0, 4, 1, 5], [2, 6, 3, 7]]     # heads per score tile
SEQIDX = {h: s for s, h in enumerate(HEADSEQ)}

NWB = 8          # w-blocks of 16 lines
WBLK = 16

# copy-engine schedules (tunable): entries are "v" (DVE), "s" (Act), "g" (Pool)
# PSUM sources: DVE/Act only (GPSIMD cannot access PSUM on real TRN2)
SPLIT_A = ["v", "s"] * 64
SPLIT_B = ["v"] * 64
# out-proj evacs are lag work: keep only ~5/64 on Act to balance engine busy.
# First 64 entries: identical cycling pattern to the balanced config; last 6
# (the epilogue, after Act's final exp) go to the otherwise-idle Act.
SPLIT_G = [(["v"] * 6 + ["s"] + ["v"] * 6)[i % 13] for i in range(64)] + ["s"] * 6
ENG_E = "v"
ENG_D = "g"


def build_program():
    nc = bacc.Bacc("TRN2")

    xw_d = nc.dram_tensor("xw_d", [128, 2, W, H], DT_B, kind="ExternalInput")
    x8_d = nc.dram_tensor("x8_d", [128, 2, W, H], DT_8, kind="ExternalInput")
    cb8_d = nc.dram_tensor("cb8_d", [128, 1024], DT_8, kind="ExternalInput")
    cb16_d = nc.dram_tensor("cb16_d", [128, 2048], DT_B, kind="ExternalInput")
    cf32_d = nc.dram_tensor("cf32_d", [128, 8], DT_F, kind="ExternalInput")
    out_d = nc.dram_tensor("out_d", [32, 2, 128, 512], DT_B, kind="ExternalOutput")

    eng = {"v": nc.vector, "s": nc.scalar, "g": nc.gpsimd}

    def copy_with_bias(e, dst, src, bias):
        if e == "s":
            nc.scalar.activation(dst, src, AF.Identity, bias=bias)
        else:
            eng[e].tensor_scalar(dst, src, bias, None, ALU.add)

    with tile.TileContext(nc) as tc:
        with (
            tc.tile_pool(name="const", bufs=1) as cpool,
            tc.tile_pool(name="xin", bufs=1) as xpool,
            tc.tile_pool(name="qk8", bufs=1) as qkpool,
            tc.tile_pool(name="vt", bufs=1) as vpool,
            tc.tile_pool(name="ex", bufs=4) as epool,
            tc.tile_pool(name="aw", bufs=4) as apool,
            tc.tile_pool(name="inv", bufs=4) as ipool,
            tc.tile_pool(name="yn", bufs=1) as ynpool,
            tc.tile_pool(name="yt", bufs=1) as ytpool,
            tc.tile_pool(name="ot", bufs=4) as otpool,
            tc.tile_pool(name="psc", bufs=2, space="PSUM") as scpool,
            tc.tile_pool(name="pst", bufs=2, space="PSUM") as stpool,
            tc.tile_pool(name="psy", bufs=1, space="PSUM") as ypool,
        ):
            # ---- constants ----
            cb8 = cpool.tile([128, 1024], DT_8, tag="cb8")
            nc.sync.dma_start(out=cb8[:], in_=cb8_d[:])
            cf = cpool.tile([128, 8], DT_F, tag="cf32")
            nc.sync.dma_start(out=cf[:], in_=cf32_d[:])
            cb16 = cpool.tile([128, 2048], DT_B, tag="cb16")
            ones = cpool.tile([128, 1], DT_B, tag="ones")
            nc.vector.memset(ones[:], 1.0)
            warm = cpool.tile([128, 1], DT_B, tag="warm")
            nc.scalar.activation(warm[:], ones[:], AF.Exp)

            w8 = cb8[:].rearrange("p (a m) -> p a m", a=2)          # [128,2,512]
            wv = cb16[:, 0:512].rearrange("p (a f) -> p a f", a=2)  # [128,2,256]
            wo = cb16[:, 512:1024].rearrange("p (a f) -> p a f", a=2)
            eb = cb16[:, 1024:2048]                                  # [m,(s,l)]

            # ---- persistent double-buffered tiles ----
            xw_t = [xpool.tile([128, 2, WBLK, H], DT_B, tag=f"xw{i}", name=f"xw{i}") for i in range(2)]
            x8_t = [xpool.tile([128, 2, WBLK, H], DT_8, tag=f"x8{i}", name=f"x8{i}") for i in range(2)]
            q8_t = [qkpool.tile([128, 2, WBLK, 128], DT_8, tag=f"q8{i}", name=f"q8{i}") for i in range(2)]
            k8_t = [qkpool.tile([128, 2, WBLK, 2, 128], DT_8, tag=f"k8{i}", name=f"k8{i}") for i in range(2)]
            nc.vector.memset(k8_t[0][:, :, :, 1, :].bitcast(mybir.dt.uint16), 0)
            nc.gpsimd.memset(k8_t[1][:, :, :, 1, :].bitcast(mybir.dt.uint16), 0)
            # V tiles: one per 4-line group, double buffered: [m, (j4, s, d)]
            v_t = [vpool.tile([128, 1024], DT_B, tag=f"v{i}", name=f"v{i}") for i in range(8)]
            yn_t = [ynpool.tile([128, 2, 4, 128], DT_B, tag=f"yn{i}", name=f"yn{i}") for i in range(4)]
            yt = ytpool.tile([128, 2, H * W], DT_B, tag="yt")

            def x_load8(wb):
                i = wb % 2
                nc.sync.dma_start(
                    out=x8_t[i][:], in_=x8_d[:, :, wb * WBLK:(wb + 1) * WBLK, :]
                )

            def x_loadw(wb):
                i = wb % 2
                nc.sync.dma_start(
                    out=xw_t[i][:], in_=xw_d[:, :, wb * WBLK:(wb + 1) * WBLK, :]
                )

            def x_load(wb):
                x_load8(wb)
                x_loadw(wb)

            # wb0 split into head (w0..4, gates pair 0) + rest
            nc.sync.dma_start(out=x8_t[0][:, :, 0:4, :], in_=x8_d[:, :, 0:4, :])
            nc.sync.dma_start(out=xw_t[0][:, :, 0:4, :], in_=xw_d[:, :, 0:4, :])
            nc.sync.dma_start(out=cb16[:], in_=cb16_d[:])
            nc.sync.dma_start(out=x8_t[0][:, :, 4:16, :], in_=x8_d[:, :, 4:16, :])
            nc.sync.dma_start(out=xw_t[0][:, :, 4:16, :], in_=xw_d[:, :, 4:16, :])
            x_load(1)

            na, nb, ng = [0], [0], [0]

            def stage_a_chunk(wb, idx):
                # idx 0..15 -> (ft, ch)
                ib = wb % 2
                ch, ft = divmod(idx, 4)
                ps = stpool.tile([128, 512], DT_F, tag="st", name="st_a")
                nc.tensor.matmul(
                    ps[:],
                    lhsT=w8[:, :, ft * 128:(ft + 1) * 128],
                    rhs=x8_t[ib][:, :, ch * 4:(ch + 1) * 4, :],
                    start=True, stop=True,
                    perf_mode=PM.DoubleRow,
                )
                if ft < 2:
                    dst = q8_t[ib][:, ft, ch * 4:(ch + 1) * 4, :]
                else:
                    dst = k8_t[ib][:, ft - 2, ch * 4:(ch + 1) * 4, 0, :]
                e = SPLIT_A[na[0] % len(SPLIT_A)]
                na[0] += 1
                copy_with_bias(e, dst, ps[:], cf[:, ft:ft + 1])

            def v_chunk(wb, idx):
                # idx 0..7 -> 2 lines
                ib = wb % 2
                vq, j2 = divmod(idx, 2)
                vt = v_t[ib * 4 + vq]
                ps = stpool.tile([128, 512], DT_F, tag="st", name="st_v")
                for j in range(2):
                    for a in range(2):
                        nc.tensor.matmul(
                            ps[:, j * 256:(j + 1) * 256],
                            lhsT=xw_t[ib][:, a, vq * 4 + j2 * 2 + j, :],
                            rhs=wv[:, a, :],
                            start=(a == 0), stop=(a == 1),
                        )
                e = SPLIT_B[nb[0] % len(SPLIT_B)]
                nb[0] += 1
                dst = vt[:, j2 * 512:(j2 + 1) * 512]
                if e == "s":
                    nc.scalar.activation(dst, ps[:], AF.Copy)
                else:
                    eng[e].tensor_copy(dst, ps[:])

            def oproj_chunk(qw, ct, half=None):
                # 4-line group qw (w = qw*4..+4), full h; one chunk per ct.
                # half=0/1: 2-line subchunk (used to shorten the tail).
                if half is None:
                    w0, nw, cols = qw * 4, 4, 512
                else:
                    w0, nw, cols = qw * 4 + half * 2, 2, 256
                ot = otpool.tile([128, 512], DT_B, tag=f"ot{ct}", name="ot")
                po = stpool.tile([128, 512], DT_F, tag="st", name="st_o")
                for a in range(2):
                    rhs = yt[:, a, :].rearrange(
                        "p (w h) -> p w h", h=H
                    )[:, w0:w0 + nw, :]
                    nc.tensor.matmul(
                        po[:, 0:cols],
                        lhsT=wo[:, a, ct * 128:(ct + 1) * 128],
                        rhs=rhs,
                        start=(a == 0), stop=(a == 1),
                    )
                e = SPLIT_G[ng[0] % len(SPLIT_G)]
                ng[0] += 1
                copy_with_bias(e, ot[:, 0:cols], po[:, 0:cols], cf[:, 4 + ct:5 + ct])
                if half is None:
                    nc.sync.dma_start(out=out_d[qw, ct], in_=ot[:])
                else:
                    nc.sync.dma_start(
                        out=out_d[qw, ct][:, half * 256:half * 256 + 256],
                        in_=ot[:, 0:256],
                    )

            def attention_pair(wb, pj):
                ib = wb % 2
                wloc = pj * 2
                sc0 = scpool.tile([128, 1024], DT_F, tag="sc", name="sc0")
                sc1 = scpool.tile([128, 1024], DT_F, tag="sc", name="sc1")
                sc = [sc0, sc1]
                attnws = []
                for tq in range(2):
                    st = sc[tq]
                    for j in range(2):
                        for qi in range(4):
                            h = QUADS[tq][qi]
                            b = h % 4
                            hh = h // 4
                            lhsT = k8_t[ib][32 * b:32 * b + 32, hh, wloc + j, :, :]
                            rhs = q8_t[ib][
                                32 * b:32 * b + 32, hh:hh + 1, wloc + j, :
                            ].broadcast_to((32, 2, 128))
                            col = (qi // 2) * 512 + (qi % 2) * 256 + j * 128
                            nc.tensor.matmul(
                                st[:, col:col + 128],
                                lhsT=lhsT, rhs=rhs,
                                start=True, stop=True,
                                perf_mode=PM.DoubleRow,
                                tile_position=(32 * b, 0),
                            )
                    exps = epool.tile([128, 1024], DT_B, tag="exps", name="exps")
                    nc.scalar.activation(exps[:], st[:], AF.Exp, scale=SCALE)
                    attnw = apool.tile([128, 1024], DT_B, tag="attnw", name="attnw")
                    e5 = exps[:].rearrange("p (s j l) -> p s j l", s=4, j=2)
                    eb5 = eb[:, tq * 512:(tq + 1) * 512].rearrange(
                        "p (s l) -> p s l", s=4
                    )[:, :, None, :]
                    i0, i1 = broadcast_tensor_aps(e5, eb5)
                    aw5 = attnw[:].rearrange("p (s j l) -> p s j l", s=4, j=2)
                    eng[ENG_D].tensor_tensor(aw5, i0, i1, ALU.mult)
                    attnws.append(attnw)

                y_ps = ypool.tile([128, 528], DT_F, tag="y", name="y_ps")
                vt = v_t[ib * 4 + pj // 2]
                jj0 = (pj % 2) * 2
                for tq in range(2):
                    attnw = attnws[tq]
                    for j in range(2):
                        for qi in range(4):
                            s = tq * 4 + qi
                            col = (qi // 2) * 512 + (qi % 2) * 256 + j * 128
                            asl = attnw[:, col:col + 128]
                            nc.tensor.matmul(
                                y_ps[:, j * 256 + s * 32:j * 256 + s * 32 + 32],
                                lhsT=asl,
                                rhs=vt[:, (jj0 + j) * 256 + s * 32:(jj0 + j) * 256 + s * 32 + 32],
                                start=True, stop=True,
                            )
                            nc.tensor.matmul(
                                y_ps[:, 512 + j * 8 + s:512 + j * 8 + s + 1],
                                lhsT=asl, rhs=ones[:],
                                start=True, stop=True,
                            )

                invd = ipool.tile([128, 16], DT_F, tag="invd", name="invd")
                nc.vector.reciprocal(invd[:], y_ps[:, 512:528])

                ynt = yn_t[(wb * 4 + pj // 2) % 4]
                y5 = y_ps[:, 0:512].rearrange(
                    "p (j ch s2 d) -> p j ch s2 d", j=2, ch=2, s2=4
                )
                iv5 = invd[:].rearrange("p (j ch s2) -> p j ch s2", j=2, ch=2)[
                    :, :, :, :, None
                ]
                out5 = ynt[:, :, (pj % 2) * 2:(pj % 2) * 2 + 2, :].rearrange(
                    "p ch j (s2 d) -> p j ch s2 d", s2=4
                )
                i0, i1 = broadcast_tensor_aps(y5, iv5)
                eng[ENG_E].tensor_tensor(out5, i0, i1, ALU.mult)

                last_grp = (wb == NWB - 1 and pj >= 6)
                if last_grp:
                    # 2-line transpose per pair so the tail drains sooner
                    lb = wb * WBLK + pj * 2
                    for chh in range(2):
                        srcv = ynt[:, chh, (pj % 2) * 2:(pj % 2) * 2 + 2, :]
                        dstv = yt[:, chh, :].rearrange(
                            "p (w h) -> p w h", h=H
                        )[:, lb:lb + 2, :]
                        nc.sync.dma_start_transpose(dstv, srcv)
                elif pj % 2 == 1:
                    lb = wb * WBLK + (pj // 2) * 4
                    for chh in range(2):
                        srcv = ynt[:, chh, :, :]
                        dstv = yt[:, chh, :].rearrange(
                            "p (w h) -> p w h", h=H
                        )[:, lb:lb + 4, :]
                        nc.sync.dma_start_transpose(dstv, srcv)

            # ---- software-pipelined emission ----
            # prologue: wb0 projections
            for i in range(16):
                stage_a_chunk(0, i)
            for i in range(8):
                v_chunk(0, i)

            for wb in range(NWB):
                for pj in range(8):
                    if pj == 0 and wb + 2 < NWB:
                        x_load8(wb + 2)
                    if pj == 2 and wb + 2 < NWB:
                        x_loadw(wb + 2)
                    # at the wb boundary (pj 0) the new wb's first scores go
                    # ahead of the prefetch matmuls so the first exp isn't
                    # delayed; mid-wb the prefetch-first order stays.
                    if wb + 1 < NWB and pj > 0:
                        stage_a_chunk(wb + 1, pj * 2)
                        stage_a_chunk(wb + 1, pj * 2 + 1)
                        v_chunk(wb + 1, pj)
                    attention_pair(wb, pj)
                    if wb + 1 < NWB and pj == 0:
                        stage_a_chunk(wb + 1, 0)
                        stage_a_chunk(wb + 1, 1)
                        v_chunk(wb + 1, 0)
                    g = wb * 8 + pj          # global pair index
                    if g >= 2:
                        oproj_chunk((g - 2) // 2, (g - 2) % 2)
            # epilogue: last group's projection, 2-line granularity
            oproj_chunk(31, 0, half=0)
            oproj_chunk(31, 1, half=0)
            oproj_chunk(31, 0, half=1)
            oproj_chunk(31, 1, half=1)

    nc.compile()
    return nc


_NC = None


def _get_nc():
    global _NC
    if _NC is None:
        _NC = build_program()
    return _NC


def _prep_small(rel_bias, Wqkv, bqkv, Wout, bout):
    rel_bias = np.asarray(rel_bias, np.float32)
    Wqkv = np.asarray(Wqkv, np.float32)
    bqkv = np.asarray(bqkv, np.float32)
    Wout = np.asarray(Wout, np.float32)
    bout = np.asarray(bout, np.float32)

    perm_v = np.concatenate([np.arange(32) + h * 32 for h in HEADSEQ])

    # cb8: qk weights as DoubleRow lhsT [p,(a,m)], features in natural order
    w8 = Wqkv[:, 0:512].reshape(2, 128, 512).transpose(1, 0, 2).reshape(128, 1024)
    cb8 = np.ascontiguousarray(w8).astype(F8)

    # cb16: [wv 512 | wo 512 | eb 1024]
    wv = Wqkv[:, 512:768][:, perm_v].reshape(2, 128, 256).transpose(1, 0, 2)
    wo = Wout[perm_v, :].reshape(2, 128, 256).transpose(1, 0, 2)
    expbt = np.exp(rel_bias.transpose(0, 2, 1))  # [hd, m, l]
    ebs = expbt[HEADSEQ].transpose(1, 0, 2).reshape(128, 1024)  # [m,(s,l)]
    cb16 = np.concatenate(
        [wv.reshape(128, 512), wo.reshape(128, 512), ebs], axis=1
    ).astype(BF16)

    bout2 = bout + bqkv[512:768] @ Wout
    cf32 = np.stack(
        [
            bqkv[0:128], bqkv[128:256], bqkv[256:384], bqkv[384:512],
            bout2[0:128], bout2[128:256],
            np.zeros(128, np.float32), np.zeros(128, np.float32),
        ],
        axis=1,
    ).astype(np.float32)

    return {
        "cb8_d": np.ascontiguousarray(cb8),
        "cb16_d": np.ascontiguousarray(cb16),
        "cf32_d": np.ascontiguousarray(cf32),
    }


def _prep_x(x_bt):
    # x_bt [C,H,W] f32 -> [p, a, w, h]
    xt = x_bt.reshape(2, 128, H, W).transpose(1, 0, 3, 2)
    xt = np.ascontiguousarray(xt)
    return xt.astype(BF16), xt.astype(F8)


def _unpack_out(arr):
    # arr [4, 2, 128, 4096] bf16 -> [C, H, W] f32
    a = np.asarray(arr).astype(np.float32).reshape(32, 2, 128, 4, 128)
    # [qw, ct, p, wl, h] -> [(ct,p)=c, h, (qw,wl)=w]
    return a.transpose(1, 2, 4, 0, 3).reshape(C, H, W)


def _run(x, rel_bias, Wqkv, bqkv, Wout, bout, **spmd_kwargs):
    x = np.asarray(x, dtype=np.float32)
    small = _prep_small(rel_bias, Wqkv, bqkv, Wout, bout)
    nc = _get_nc()
    core_ids = list(range(8))
    in_maps = []
    for i in core_ids:
        b, t = divmod(i, T)
        xw, x8 = _prep_x(x[b, t])
        m = dict(small)
        m["xw_d"] = xw
        m["x8_d"] = x8
        in_maps.append(m)
    res = run_bass_kernel_spmd(nc, in_maps, core_ids, **spmd_kwargs)
    out = np.empty((B, T, C, H, W), np.float32)
    for i in core_ids:
        b, t = divmod(i, T)
        out[b, t] = _unpack_out(res.results[i]["out_d"])
    return out, res


def kernel(x, rel_bias, Wqkv, bqkv, Wout, bout):
    out, _ = _run(x, rel_bias, Wqkv, bqkv, Wout, bout)
    return out

